# revision 26
# baseline (speedup 1.0000x reference)
"""Trainium2 Bass kernel for nn_BartCrossAttention (B=4, L=1024, D=1024, H=16, HD=64).

Sharding: 8 cores; core c handles query tokens [512c, 512c+512) (batch b = c//2).
Each core recomputes K/V projections for its *whole* batch (1024 kv tokens) so no
collective is needed (a pairwise K/V exchange was measured at ~90us through this
stack's DRAM AllGather — far too slow to hide); the host slices inputs per core
and concatenates outputs.

Key design points (vs the fp32r v1 baseline, 264.6us -> ~200-205us):
- Everything bf16 (PE rate is 1 cycle/row for bf16 and for fp32r at free>=256,
  but bf16 halves DMA, SBUF, and DVE traffic); fp32 PSUM accumulation and an
  fp32 softmax chain keep rel err ~6e-3 (gate 2e-2).
- hid^T / kv^T are transposed on the HOST, so the PE does no transposes; the
  kvT/wv DMAs are split in halves and interleaved so the V projection starts
  as soon as the first halves land.
- All four weights live in SBUF whole; K/Q projections for pair p+1 are
  interleaved into pair p's attention t-loop.
- Fused softmax denominators ride the ctx matmul as a 65th lhsT column of ones.
  Per-pair normalization runs fully in SBUF: DVE copies the two denominator
  rows out of PSUM, DVE reciprocal (6.5us on a 1-partition row, but nothing
  the PE waits on sits behind it: the K/Q PSUM evicts were moved to ACT as
  Identity+per-partition-bias), gpsimd broadcasts down 64 partitions, DVE
  multiplies fused with the PSUM evict. psctx has 4 banks so consecutive
  pairs double-buffer and the chain hides under the next pair's attention.
- PSUM: main loop = 4 ctx banks + 4 score/proj banks (per-tag pools); the
  epilogue reuses the score pool (a fresh pool would barrier on the last norm
  chain). Out-projection runs fj=0..6 of all 8 output chains first (partials
  evicted to SBUF with the bias folded in, freeing banks), then the 8 fj=7
  rank-128 updates after the last pair's norm resolves.
- exp runs per [128,512] score half on ACT, double-buffered against the PE.
"""
import sys

for _p in ("/opt/trn_rl_repo",):
    if _p not in sys.path:
        sys.path.insert(0, _p)

import numpy as np
import ml_dtypes

import concourse.bass as bass
import concourse.mybir as mybir
import concourse.tile as tile
from concourse import bacc
import concourse.bass_utils as bass_utils

F32 = mybir.dt.float32
BF16 = mybir.dt.bfloat16

P = 128
D = 1024        # model dim
H = 16          # heads
NCORES = 8
TQ = 512        # query tokens per core
LK = 1024       # kv tokens per batch
B, LQ = 4, 1024

_CACHE = {}


def _build_core_program():
    nc = bacc.Bacc("TRN2", target_bir_lowering=False, debug=False,
                   num_devices=NCORES)

    hid_s = nc.dram_tensor("hid_s", [D, TQ], BF16, kind="ExternalInput")
    kv_s = nc.dram_tensor("kv_s", [D, LK], BF16, kind="ExternalInput")
    wq_t = nc.dram_tensor("wq_t", [D, D], BF16, kind="ExternalInput")
    wk_t = nc.dram_tensor("wk_t", [D, D], BF16, kind="ExternalInput")
    wv_t = nc.dram_tensor("wv_t", [D, D], BF16, kind="ExternalInput")
    wo_t = nc.dram_tensor("wo_t", [D, D], BF16, kind="ExternalInput")
    qb_d = nc.dram_tensor("qb", [D], F32, kind="ExternalInput")
    kb_d = nc.dram_tensor("kb", [D], F32, kind="ExternalInput")
    vb_d = nc.dram_tensor("vb", [D], F32, kind="ExternalInput")
    ob_d = nc.dram_tensor("ob", [D], F32, kind="ExternalInput")
    out_s = nc.dram_tensor("out_s", [TQ, D], F32, kind="ExternalOutput")

    Exp = mybir.ActivationFunctionType.Exp
    Ident = mybir.ActivationFunctionType.Identity
    add = mybir.AluOpType.add
    mult = mybir.AluOpType.mult

    with tile.TileContext(nc) as tc:
        with (
            tc.tile_pool(name="setup", bufs=1) as setup,
            tc.tile_pool(name="big", bufs=1) as big,
            tc.tile_pool(name="attn", bufs=6) as attnp,
            tc.tile_pool(name="wfull", bufs=1) as wfull,
            tc.tile_pool(name="rb", bufs=1) as rbp,
            tc.tile_pool(name="outp", bufs=4) as outp,
            tc.tile_pool(name="partp", bufs=8) as partp,
        ):
            # ---- setup: biases ----
            qb_sb = setup.tile([P, 8], F32, tag="qb")
            nc.sync.dma_start(qb_sb[:], qb_d.ap().rearrange("(o p) -> p o", p=P))
            kb_sb = setup.tile([P, 8], F32, tag="kb")
            nc.sync.dma_start(kb_sb[:], kb_d.ap().rearrange("(o p) -> p o", p=P))
            vbB = setup.tile([P, D], F32, tag="vbB")
            obB = setup.tile([P, D], F32, tag="obB")

            def load_w_full(dram, tag):
                # [D, D] -> tile [128, 8, 1024]: w[di*128+p, o] at [p, di, o]
                t = wfull.tile([P, 8, D], BF16, tag=tag)
                nc.sync.dma_start(
                    t[:], dram.ap().rearrange("(dd p) o -> p dd o", p=P))
                return t

            # ---- persistent big tiles ----
            KT = big.tile([P, 8, LK], BF16, tag="KT")        # K^T [1024, 1024]
            v65 = big.tile([P, 8, H * 65], BF16, tag="v65")  # V+ones [1024,1040]
            qT = big.tile([P, 8, TQ], BF16, tag="qT")        # Q^T [1024, 512]
            ctxT = big.tile([P, 8, TQ], BF16, tag="ctxT")    # ctx^T [1024, 512]

            # ones columns of v65 (col 64 of each head block)
            onesF = setup.tile([P, P], F32, tag="onesF")
            nc.gpsimd.memset(onesF[:], 1.0)
            nc.vector.tensor_copy(
                v65[:].rearrange("p t (h x) -> p t h x", x=65)[:, :, :, 64:65],
                onesF[:].rearrange("p (t h x) -> p t h x", t=8, h=16))

            with tc.tile_pool(name="xTp", bufs=1) as xTp:
                kvT = xTp.tile([P, 8, LK], BF16, tag="kvT")   # kv^T [D, 1024]
                hidT = xTp.tile([P, 8, TQ], BF16, tag="hidT")  # hid^T [1024,512]

                with (
                    tc.tile_pool(name="browp", bufs=2) as browp,
                    tc.tile_pool(name="wvpool", bufs=1) as wvpool,
                    tc.tile_pool(name="psv", bufs=2, space="PSUM") as psv,
                ):
                    # bias rows -> broadcast
                    vb_row = browp.tile([1, D], F32, tag="brow")
                    nc.sync.dma_start(vb_row[:], vb_d.ap()[None, :])
                    nc.gpsimd.partition_broadcast(vbB[:], vb_row[:])
                    ob_row = browp.tile([1, D], F32, tag="brow")
                    nc.sync.dma_start(ob_row[:], ob_d.ap()[None, :])
                    nc.gpsimd.partition_broadcast(obB[:], ob_row[:])

                    # ---- prologue: kv^T and hid^T arrive pre-transposed
                    # from the host; kvT token-halves interleave with the wv
                    # halves on the DMA queue so V proj starts as soon as
                    # the first halves of each land ----
                    kvt_ap = kv_s.ap().rearrange("(dd p) t -> p dd t", p=P)
                    wv_ap = wv_t.ap().rearrange("(dd p) o -> p dd o", p=P)
                    wv_halves = []
                    # first V-proj matmul only needs kv tokens 0:256 and the
                    # first contraction half of wv's first column-half, so
                    # split those DMAs fine-grained to shrink the gate.
                    nc.sync.dma_start(kvT[:, :, 0:256], kvt_ap[:, :, 0:256])
                    wvh = wvpool.tile([P, 8, 512], BF16, tag="wv")
                    nc.sync.dma_start(wvh[:, 0:4, :], wv_ap[:, 0:4, 0:512])
                    nc.sync.dma_start(wvh[:, 4:8, :], wv_ap[:, 4:8, 0:512])
                    wv_halves.append(wvh)
                    nc.sync.dma_start(kvT[:, :, 256:512],
                                      kvt_ap[:, :, 256:512])
                    # hidT next: Q-proj for pair 0 fires right after the V
                    # projection and must not wait on the activation stream
                    nc.sync.dma_start(
                        hidT[:],
                        hid_s.ap().rearrange("(dd p) t -> p dd t", p=P))
                    nc.sync.dma_start(kvT[:, :, 512:768],
                                      kvt_ap[:, :, 512:768])
                    wvh = wvpool.tile([P, 8, 512], BF16, tag="wv")
                    nc.sync.dma_start(wvh[:], wv_ap[:, :, 512:1024])
                    wv_halves.append(wvh)
                    nc.sync.dma_start(kvT[:, :, 768:1024],
                                      kvt_ap[:, :, 768:1024])

                    for half in range(2):             # v-col half
                        wv = wv_halves[half]
                        for ti in range(8):           # kv token tile
                            pp = psv.tile([P, 512], F32, tag="pp")
                            for di in range(8):
                                nc.tensor.matmul(
                                    pp[:],
                                    kvT[:, di, ti * P:(ti + 1) * P],
                                    wv[:, di, :],
                                    start=(di == 0), stop=(di == 7),
                                )
                            dst = v65[:].rearrange(
                                "p t (h x) -> p t h x", x=65)[
                                :, ti, half * 8:(half + 1) * 8, 0:64]
                            nc.vector.tensor_tensor(
                                dst, pp[:],
                                vbB[:, half * 512:(half + 1) * 512], add)

                # ---- main loop: per head-pair K/Q projection + attention ----
                with (
                    tc.tile_pool(name="psctx", bufs=4, space="PSUM") as psctx,
                    tc.tile_pool(name="pssc", bufs=4, space="PSUM") as pssc,
                ):
                    psk = pssc
                    wk = load_w_full(wk_t, "wk")
                    wq = load_w_full(wq_t, "wq")
                    wo = load_w_full(wo_t, "wo")

                    def emit_kproj(hp, nk):
                        pp = psk.tile([P, 512], F32, tag="sc",
                                      name=f"ppk{hp}_{nk}")
                        for di in range(8):
                            nc.tensor.matmul(
                                pp[:],
                                wk[:, di, hp * P:(hp + 1) * P],
                                kvT[:, di, nk * 512:(nk + 1) * 512],
                                start=(di == 0), stop=(di == 7),
                            )
                        nc.scalar.activation(
                            KT[:, hp, nk * 512:(nk + 1) * 512], pp[:],
                            Ident, bias=kb_sb[:, hp:hp + 1])

                    def emit_qproj(hp):
                        pq = psk.tile([P, 512], F32, tag="sc",
                                      name=f"ppq{hp}")
                        for di in range(8):
                            nc.tensor.matmul(
                                pq[:],
                                wq[:, di, hp * P:(hp + 1) * P],
                                hidT[:, di, :],
                                start=(di == 0), stop=(di == 7),
                            )
                        nc.scalar.activation(qT[:, hp, :], pq[:],
                                             Ident, bias=qb_sb[:, hp:hp + 1])

                    # pair 0 projections up front (t<4 only needs nk=0)
                    emit_kproj(0, 0)
                    emit_qproj(0)
                    emit_kproj(0, 1)

                    for hp in range(8):
                        nxt = hp + 1
                        ctx_ps = [psctx.tile([65, 512], F32, tag="ctx",
                                             name=f"ctx{hp}_{i}")
                                  for i in range(2)]

                        def emit_ctx(t, ats):
                            for hh in range(2):
                                h = 2 * hp + hh
                                nc.tensor.matmul(
                                    ctx_ps[hh][:],
                                    v65[:, t, h * 65:(h + 1) * 65],
                                    ats[hh][:],
                                    start=(t == 0), stop=(t == 7),
                                )

                        # software pipeline: scores/exp for tile t issue
                        # before the ctx matmuls of tile t-1, so each exp
                        # hides behind the next score matmul's streaming.
                        prev = None
                        for t in range(8):
                            ats = []
                            for hh in range(2):
                                lo = 64 * hh
                                sc = pssc.tile([P, 512], F32, tag="sc",
                                               name=f"sc{hp}_{t}_{hh}")
                                nc.tensor.matmul(
                                    sc[:],
                                    KT[lo:lo + 64, hp, t * P:(t + 1) * P],
                                    qT[lo:lo + 64, hp, :],
                                    start=True, stop=True,
                                )
                                at = attnp.tile([P, 512], BF16, tag="at")
                                nc.scalar.activation(at[:], sc[:], Exp)
                                ats.append(at)
                            if prev is not None:
                                emit_ctx(t - 1, prev)
                            prev = ats
                            if nxt < 8:
                                if t == 1:
                                    emit_kproj(nxt, 0)
                                elif t == 3:
                                    emit_kproj(nxt, 1)
                                elif t == 5:
                                    emit_qproj(nxt)
                        emit_ctx(7, prev)
                        # per-pair softmax normalization, all in SBUF:
                        # DVE copies the denominator rows out of PSUM, ACT
                        # takes the reciprocal (free-size bound), gpsimd
                        # broadcasts down 64 partitions, DVE multiplies
                        # fused with the PSUM evict.
                        # DVE reciprocal costs 6.5us on a 1-partition row,
                        # but with the K/Q evicts moved to ACT nothing the PE
                        # waits on sits behind it on the vector queue, and
                        # psctx bufs=4 gives the chain two pair-periods of
                        # slack before its PSUM banks are needed again.
                        srow = rbp.tile([1, 1024], F32, tag="srow",
                                        name=f"srow{hp}")
                        rrow = rbp.tile([1, 1024], F32, tag="rrow",
                                        name=f"rrow{hp}")
                        nc.vector.tensor_copy(srow[:, 0:512],
                                              ctx_ps[0][64:65, :])
                        nc.vector.tensor_copy(srow[:, 512:1024],
                                              ctx_ps[1][64:65, :])
                        nc.vector.reciprocal(rrow[:], srow[:])
                        rcp = rbp.tile([64, 1024], F32, tag="rcp",
                                       name=f"rcp{hp}")
                        nc.gpsimd.partition_broadcast(rcp[:], rrow[:])
                        nc.vector.tensor_tensor(
                            ctxT[0:64, hp, :], ctx_ps[0][0:64, :],
                            rcp[:, 0:512], mult)
                        nc.vector.tensor_tensor(
                            ctxT[64:128, hp, :], ctx_ps[1][0:64, :],
                            rcp[:, 512:1024], mult)

                    # ---- epilogue: out projection (inside the main PSUM
                    # scope: po accumulators come from pssc, whose banks free
                    # fast — a fresh pool here would barrier on the last
                    # pair's norm chain). Each chain's fj=7 lhsT (pair 7's
                    # ctxT) is only ready after that ~10us chain; running
                    # fj=0..6 of several chains ahead of any fj=7 hides it.
                    chains = [(half, mi) for half in range(2)
                              for mi in range(4)]

                    # fj=0..6 of every chain accumulates first (partials
                    # evicted to SBUF with the bias folded in, freeing the
                    # PSUM bank); the fj=7 rank-128 updates and final adds
                    # run after the last pair's norm chain resolves.
                    partials = []
                    for ci, (half, mi) in enumerate(chains):
                        po = pssc.tile([P, 512], F32, tag="sc",
                                       name=f"po{ci}")
                        for fj in range(7):
                            nc.tensor.matmul(
                                po[:],
                                ctxT[:, fj, mi * P:(mi + 1) * P],
                                wo[:, fj, half * 512:(half + 1) * 512],
                                start=(fj == 0), stop=(fj == 6),
                            )
                        part = partp.tile([P, 512], BF16, tag="part",
                                          name=f"part{ci}")
                        nc.vector.tensor_tensor(
                            part[:], po[:],
                            obB[:, half * 512:(half + 1) * 512], add)
                        partials.append(part)
                    for ci, (half, mi) in enumerate(chains):
                        po7 = pssc.tile([P, 512], F32, tag="sc",
                                        name=f"po7_{ci}")
                        nc.tensor.matmul(
                            po7[:],
                            ctxT[:, 7, mi * P:(mi + 1) * P],
                            wo[:, 7, half * 512:(half + 1) * 512],
                            start=True, stop=True,
                        )
                        ot = outp.tile([P, 512], F32, tag="ot")
                        nc.vector.tensor_tensor(
                            ot[:], po7[:], partials[ci][:], add)
                        nc.sync.dma_start(
                            out_s.ap().rearrange("(mm p) d -> p mm d", p=P)[
                                :, mi, half * 512:(half + 1) * 512],
                            ot[:])

    nc.compile()
    return nc


def _prep_inputs(hidden_states, key_value_states, q_weight, q_bias,
                 kv_weight, kv_bias, out_weight, out_bias):
    f32 = np.float32
    bf16 = ml_dtypes.bfloat16
    hid = np.asarray(hidden_states, f32).reshape(B * LQ, D).astype(bf16)
    kv = np.asarray(key_value_states, f32).reshape(B * LK, D).astype(bf16)
    hidT_cores = [np.ascontiguousarray(hid[c * TQ:(c + 1) * TQ].T)
                  for c in range(NCORES)]
    kvT_batches = [np.ascontiguousarray(kv[b * LK:(b + 1) * LK].T)
                   for b in range(B)]
    scale = f32(1.0 / 8.0)

    # de-interleave kv rows: row e <-> (h=e//128, j=(e%128)//64, d=e%64)
    e = np.arange(2 * D)
    kmask = (e % 128) < 64
    kidx, vidx = e[kmask], e[~kmask]
    kvw = np.asarray(kv_weight, f32)
    kvb = np.asarray(kv_bias, f32)

    shared = {
        "wq_t": np.ascontiguousarray(
            (np.asarray(q_weight, f32) * scale).T).astype(bf16),
        "wk_t": np.ascontiguousarray(kvw[kidx].T).astype(bf16),
        "wv_t": np.ascontiguousarray(kvw[vidx].T).astype(bf16),
        "wo_t": np.ascontiguousarray(np.asarray(out_weight, f32).T).astype(bf16),
        "qb": np.ascontiguousarray(np.asarray(q_bias, f32) * scale),
        "kb": np.ascontiguousarray(kvb[kidx]),
        "vb": np.ascontiguousarray(kvb[vidx]),
        "ob": np.ascontiguousarray(np.asarray(out_bias, f32)),
    }
    in_maps = []
    for c in range(NCORES):
        b = c // 2
        m = dict(shared)
        m["hid_s"] = hidT_cores[c]
        m["kv_s"] = kvT_batches[b]
        in_maps.append(m)
    return in_maps


def kernel(hidden_states, key_value_states, q_weight, q_bias,
           kv_weight, kv_bias, out_weight, out_bias, _trace=False):
    if "nc" not in _CACHE:
        _CACHE["nc"] = _build_core_program()
    nc = _CACHE["nc"]
    in_maps = _prep_inputs(hidden_states, key_value_states, q_weight, q_bias,
                           kv_weight, kv_bias, out_weight, out_bias)
    res = bass_utils.run_bass_kernel_spmd(
        nc, in_maps, core_ids=list(range(NCORES)), trace=_trace)
    _CACHE["last_result"] = res
    out = np.concatenate([r["out_s"] for r in res.results], axis=0)
    return out.reshape(B, LQ, D)


# revision 27
# speedup vs baseline: 1.0437x; 1.0437x over previous
"""Trainium2 Bass kernel for nn_BartCrossAttention (B=4, L=1024, D=1024, H=16, HD=64).

Sharding: 8 cores; core c handles query tokens [512c, 512c+512) (batch b = c//2).
Each core recomputes K/V projections for its *whole* batch (1024 kv tokens) so no
collective is needed (a pairwise K/V exchange was measured at ~90us through this
stack's DRAM AllGather — far too slow to hide); the host slices inputs per core
and concatenates outputs.

Key design points (vs the fp32r v1 baseline, 264.6us -> ~200-205us):
- Everything bf16 (PE rate is 1 cycle/row for bf16 and for fp32r at free>=256,
  but bf16 halves DMA, SBUF, and DVE traffic); fp32 PSUM accumulation and an
  fp32 softmax chain keep rel err ~6e-3 (gate 2e-2).
- hid^T / kv^T are transposed on the HOST, so the PE does no transposes; the
  kvT/wv DMAs are split in halves and interleaved so the V projection starts
  as soon as the first halves land.
- All four weights live in SBUF whole; K/Q projections for pair p+1 are
  interleaved into pair p's attention t-loop.
- Fused softmax denominators ride the ctx matmul as a 65th lhsT column of ones.
  Per-pair normalization runs fully in SBUF: DVE copies the two denominator
  rows out of PSUM, DVE reciprocal (6.5us on a 1-partition row, but nothing
  the PE waits on sits behind it: the K/Q PSUM evicts were moved to ACT as
  Identity+per-partition-bias), gpsimd broadcasts down 64 partitions, DVE
  multiplies fused with the PSUM evict. psctx has 4 banks so consecutive
  pairs double-buffer and the chain hides under the next pair's attention.
- PSUM: main loop = 4 ctx banks + 4 score/proj banks (per-tag pools); the
  epilogue reuses the score pool (a fresh pool would barrier on the last norm
  chain). Out-projection runs fj=0..6 of all 8 output chains first (partials
  evicted to SBUF with the bias folded in, freeing banks), then the 8 fj=7
  rank-128 updates after the last pair's norm resolves.
- exp runs per [128,512] score half on ACT, double-buffered against the PE.
"""
import sys

for _p in ("/opt/trn_rl_repo",):
    if _p not in sys.path:
        sys.path.insert(0, _p)

import numpy as np
import ml_dtypes

import concourse.bass as bass
import concourse.mybir as mybir
import concourse.tile as tile
from concourse import bacc
import concourse.bass_utils as bass_utils

F32 = mybir.dt.float32
BF16 = mybir.dt.bfloat16

P = 128
D = 1024        # model dim
H = 16          # heads
NCORES = 8
TQ = 512        # query tokens per core
LK = 1024       # kv tokens per batch
B, LQ = 4, 1024

_CACHE = {}


def _build_core_program():
    nc = bacc.Bacc("TRN2", target_bir_lowering=False, debug=False,
                   num_devices=NCORES)

    hid_s = nc.dram_tensor("hid_s", [D, TQ], BF16, kind="ExternalInput")
    kv_s = nc.dram_tensor("kv_s", [D, LK], BF16, kind="ExternalInput")
    wq_t = nc.dram_tensor("wq_t", [D, D], BF16, kind="ExternalInput")
    wk_t = nc.dram_tensor("wk_t", [D, D], BF16, kind="ExternalInput")
    wv_t = nc.dram_tensor("wv_t", [D, D], BF16, kind="ExternalInput")
    wo_t = nc.dram_tensor("wo_t", [D, D], BF16, kind="ExternalInput")
    qb_d = nc.dram_tensor("qb", [D], F32, kind="ExternalInput")
    kb_d = nc.dram_tensor("kb", [D], F32, kind="ExternalInput")
    vb_d = nc.dram_tensor("vb", [D], F32, kind="ExternalInput")
    ob_d = nc.dram_tensor("ob", [D], F32, kind="ExternalInput")
    out_s = nc.dram_tensor("out_s", [TQ, D], F32, kind="ExternalOutput")

    Exp = mybir.ActivationFunctionType.Exp
    Ident = mybir.ActivationFunctionType.Identity
    add = mybir.AluOpType.add
    mult = mybir.AluOpType.mult

    with tile.TileContext(nc) as tc:
        with (
            tc.tile_pool(name="setup", bufs=1) as setup,
            tc.tile_pool(name="big", bufs=1) as big,
            tc.tile_pool(name="attn", bufs=6) as attnp,
            tc.tile_pool(name="wfull", bufs=1) as wfull,
            tc.tile_pool(name="rb", bufs=1) as rbp,
            tc.tile_pool(name="outp", bufs=4) as outp,
            tc.tile_pool(name="partp", bufs=8) as partp,
        ):
            # ---- setup: biases ----
            qb_sb = setup.tile([P, 8], F32, tag="qb")
            nc.sync.dma_start(qb_sb[:], qb_d.ap().rearrange("(o p) -> p o", p=P))
            kb_sb = setup.tile([P, 8], F32, tag="kb")
            nc.sync.dma_start(kb_sb[:], kb_d.ap().rearrange("(o p) -> p o", p=P))
            vbB = setup.tile([P, D], F32, tag="vbB")
            obB = setup.tile([P, D], F32, tag="obB")

            def load_w_full(dram, tag):
                # [D, D] -> tile [128, 8, 1024]: w[di*128+p, o] at [p, di, o]
                t = wfull.tile([P, 8, D], BF16, tag=tag)
                nc.sync.dma_start(
                    t[:], dram.ap().rearrange("(dd p) o -> p dd o", p=P))
                return t

            # ---- persistent big tiles ----
            KT = big.tile([P, 8, LK], BF16, tag="KT")        # K^T [1024, 1024]
            v65 = big.tile([P, 8, H * 65], BF16, tag="v65")  # V+ones [1024,1040]
            qT = big.tile([P, 8, TQ], BF16, tag="qT")        # Q^T [1024, 512]
            ctxT = big.tile([P, 8, TQ], BF16, tag="ctxT")    # ctx^T [1024, 512]

            # ones columns of v65 (col 64 of each head block)
            onesF = setup.tile([P, P], F32, tag="onesF")
            nc.gpsimd.memset(onesF[:], 1.0)
            nc.vector.tensor_copy(
                v65[:].rearrange("p t (h x) -> p t h x", x=65)[:, :, :, 64:65],
                onesF[:].rearrange("p (t h x) -> p t h x", t=8, h=16))

            with tc.tile_pool(name="xTp", bufs=1) as xTp:
                kvT = xTp.tile([P, 8, LK], BF16, tag="kvT")   # kv^T [D, 1024]
                hidT = xTp.tile([P, 8, TQ], BF16, tag="hidT")  # hid^T [1024,512]

                with (
                    tc.tile_pool(name="browp", bufs=2) as browp,
                    tc.tile_pool(name="wvpool", bufs=1) as wvpool,
                    tc.tile_pool(name="psv", bufs=2, space="PSUM") as psv,
                ):
                    # bias rows -> broadcast
                    vb_row = browp.tile([1, D], F32, tag="brow")
                    nc.sync.dma_start(vb_row[:], vb_d.ap()[None, :])
                    nc.gpsimd.partition_broadcast(vbB[:], vb_row[:])
                    ob_row = browp.tile([1, D], F32, tag="brow")
                    nc.sync.dma_start(ob_row[:], ob_d.ap()[None, :])
                    nc.gpsimd.partition_broadcast(obB[:], ob_row[:])

                    # ---- prologue: kv^T and hid^T arrive pre-transposed
                    # from the host; kvT token-halves interleave with the wv
                    # halves on the DMA queue so V proj starts as soon as
                    # the first halves of each land ----
                    kvt_ap = kv_s.ap().rearrange("(dd p) t -> p dd t", p=P)
                    wv_ap = wv_t.ap().rearrange("(dd p) o -> p dd o", p=P)
                    wv_halves = []
                    # first V-proj matmul only needs kv tokens 0:256 and the
                    # first contraction half of wv's first column-half, so
                    # split those DMAs fine-grained to shrink the gate.
                    nc.sync.dma_start(kvT[:, :, 0:256], kvt_ap[:, :, 0:256])
                    wvh = wvpool.tile([P, 8, 512], BF16, tag="wv")
                    nc.sync.dma_start(wvh[:, 0:4, :], wv_ap[:, 0:4, 0:512])
                    nc.sync.dma_start(wvh[:, 4:8, :], wv_ap[:, 4:8, 0:512])
                    wv_halves.append(wvh)
                    nc.sync.dma_start(kvT[:, :, 256:512],
                                      kvt_ap[:, :, 256:512])
                    # hidT next: Q-proj for pair 0 fires right after the V
                    # projection and must not wait on the activation stream
                    nc.sync.dma_start(
                        hidT[:],
                        hid_s.ap().rearrange("(dd p) t -> p dd t", p=P))
                    nc.sync.dma_start(kvT[:, :, 512:768],
                                      kvt_ap[:, :, 512:768])
                    wvh = wvpool.tile([P, 8, 512], BF16, tag="wv")
                    nc.sync.dma_start(wvh[:], wv_ap[:, :, 512:1024])
                    wv_halves.append(wvh)
                    nc.sync.dma_start(kvT[:, :, 768:1024],
                                      kvt_ap[:, :, 768:1024])

                    for half in range(2):             # v-col half
                        wv = wv_halves[half]
                        for ti in range(8):           # kv token tile
                            pp = psv.tile([P, 512], F32, tag="pp")
                            for di in range(8):
                                nc.tensor.matmul(
                                    pp[:],
                                    kvT[:, di, ti * P:(ti + 1) * P],
                                    wv[:, di, :],
                                    start=(di == 0), stop=(di == 7),
                                )
                            dst = v65[:].rearrange(
                                "p t (h x) -> p t h x", x=65)[
                                :, ti, half * 8:(half + 1) * 8, 0:64]
                            nc.vector.tensor_tensor(
                                dst, pp[:],
                                vbB[:, half * 512:(half + 1) * 512], add)

                # ---- main loop: per head-pair K/Q projection + attention ----
                with (
                    tc.tile_pool(name="psctx", bufs=4, space="PSUM") as psctx,
                    tc.tile_pool(name="pssc", bufs=4, space="PSUM") as pssc,
                ):
                    psk = pssc
                    # pair-0/1 columns of wk/wq land first so the pair-0
                    # projections never wait on the bulk of the weights
                    def load_w_split(dram, tag):
                        t = wfull.tile([P, 8, D], BF16, tag=tag)
                        ap = dram.ap().rearrange("(dd p) o -> p dd o", p=P)
                        nc.sync.dma_start(t[:, :, 0:256], ap[:, :, 0:256])
                        return t, ap

                    wk, wk_ap = load_w_split(wk_t, "wk")
                    wq, wq_ap = load_w_split(wq_t, "wq")
                    nc.sync.dma_start(wk[:, :, 256:1024], wk_ap[:, :, 256:1024])
                    nc.sync.dma_start(wq[:, :, 256:1024], wq_ap[:, :, 256:1024])
                    wo = load_w_full(wo_t, "wo")

                    def emit_kproj(hp, nk):
                        pp = psk.tile([P, 512], F32, tag="sc",
                                      name=f"ppk{hp}_{nk}")
                        for di in range(8):
                            nc.tensor.matmul(
                                pp[:],
                                wk[:, di, hp * P:(hp + 1) * P],
                                kvT[:, di, nk * 512:(nk + 1) * 512],
                                start=(di == 0), stop=(di == 7),
                            )
                        nc.scalar.activation(
                            KT[:, hp, nk * 512:(nk + 1) * 512], pp[:],
                            Ident, bias=kb_sb[:, hp:hp + 1])

                    def emit_qproj(hp):
                        pq = psk.tile([P, 512], F32, tag="sc",
                                      name=f"ppq{hp}")
                        for di in range(8):
                            nc.tensor.matmul(
                                pq[:],
                                wq[:, di, hp * P:(hp + 1) * P],
                                hidT[:, di, :],
                                start=(di == 0), stop=(di == 7),
                            )
                        nc.scalar.activation(qT[:, hp, :], pq[:],
                                             Ident, bias=qb_sb[:, hp:hp + 1])

                    # pair 0 projections up front (t<4 only needs nk=0)
                    emit_kproj(0, 0)
                    emit_qproj(0)
                    emit_kproj(0, 1)

                    for hp in range(8):
                        nxt = hp + 1
                        ctx_ps = [psctx.tile([65, 512], F32, tag="ctx",
                                             name=f"ctx{hp}_{i}")
                                  for i in range(2)]

                        def emit_ctx(t, ats):
                            for hh in range(2):
                                h = 2 * hp + hh
                                nc.tensor.matmul(
                                    ctx_ps[hh][:],
                                    v65[:, t, h * 65:(h + 1) * 65],
                                    ats[hh][:],
                                    start=(t == 0), stop=(t == 7),
                                )

                        # software pipeline: scores/exp for tile t issue
                        # before the ctx matmuls of tile t-1, so each exp
                        # hides behind the next score matmul's streaming.
                        prev = None
                        for t in range(8):
                            ats = []
                            for hh in range(2):
                                lo = 64 * hh
                                sc = pssc.tile([P, 512], F32, tag="sc",
                                               name=f"sc{hp}_{t}_{hh}")
                                nc.tensor.matmul(
                                    sc[:],
                                    KT[lo:lo + 64, hp, t * P:(t + 1) * P],
                                    qT[lo:lo + 64, hp, :],
                                    start=True, stop=True,
                                )
                                at = attnp.tile([P, 512], BF16, tag="at")
                                nc.scalar.activation(at[:], sc[:], Exp)
                                ats.append(at)
                            if prev is not None:
                                emit_ctx(t - 1, prev)
                            prev = ats
                            if nxt < 8:
                                if t == 1:
                                    emit_kproj(nxt, 0)
                                elif t == 3:
                                    emit_kproj(nxt, 1)
                                elif t == 5:
                                    emit_qproj(nxt)
                        emit_ctx(7, prev)
                        # per-pair softmax normalization, all in SBUF:
                        # DVE copies the denominator rows out of PSUM, ACT
                        # takes the reciprocal (free-size bound), gpsimd
                        # broadcasts down 64 partitions, DVE multiplies
                        # fused with the PSUM evict.
                        # DVE reciprocal costs 6.5us on a 1-partition row,
                        # but with the K/Q evicts moved to ACT nothing the PE
                        # waits on sits behind it on the vector queue, and
                        # psctx bufs=4 gives the chain two pair-periods of
                        # slack before its PSUM banks are needed again.
                        srow = rbp.tile([1, 1024], F32, tag="srow",
                                        name=f"srow{hp}")
                        rrow = rbp.tile([1, 1024], F32, tag="rrow",
                                        name=f"rrow{hp}")
                        nc.vector.tensor_copy(srow[:, 0:512],
                                              ctx_ps[0][64:65, :])
                        nc.vector.tensor_copy(srow[:, 512:1024],
                                              ctx_ps[1][64:65, :])
                        nc.vector.reciprocal(rrow[:], srow[:])
                        rcp = rbp.tile([64, 1024], F32, tag="rcp",
                                       name=f"rcp{hp}")
                        nc.gpsimd.partition_broadcast(rcp[:], rrow[:])
                        nc.vector.tensor_tensor(
                            ctxT[0:64, hp, :], ctx_ps[0][0:64, :],
                            rcp[:, 0:512], mult)
                        nc.vector.tensor_tensor(
                            ctxT[64:128, hp, :], ctx_ps[1][0:64, :],
                            rcp[:, 512:1024], mult)

                    # ---- epilogue: out projection (inside the main PSUM
                    # scope: po accumulators come from pssc, whose banks free
                    # fast — a fresh pool here would barrier on the last
                    # pair's norm chain). Each chain's fj=7 lhsT (pair 7's
                    # ctxT) is only ready after that ~10us chain; running
                    # fj=0..6 of several chains ahead of any fj=7 hides it.
                    chains = [(half, mi) for half in range(2)
                              for mi in range(4)]

                    # fj=0..6 of every chain accumulates first (partials
                    # evicted to SBUF with the bias folded in, freeing the
                    # PSUM bank); the fj=7 rank-128 updates and final adds
                    # run after the last pair's norm chain resolves.
                    partials = []
                    for ci, (half, mi) in enumerate(chains):
                        po = pssc.tile([P, 512], F32, tag="sc",
                                       name=f"po{ci}")
                        for fj in range(7):
                            nc.tensor.matmul(
                                po[:],
                                ctxT[:, fj, mi * P:(mi + 1) * P],
                                wo[:, fj, half * 512:(half + 1) * 512],
                                start=(fj == 0), stop=(fj == 6),
                            )
                        part = partp.tile([P, 512], BF16, tag="part",
                                          name=f"part{ci}")
                        nc.vector.tensor_tensor(
                            part[:], po[:],
                            obB[:, half * 512:(half + 1) * 512], add)
                        partials.append(part)
                    for ci, (half, mi) in enumerate(chains):
                        po7 = pssc.tile([P, 512], F32, tag="sc",
                                        name=f"po7_{ci}")
                        nc.tensor.matmul(
                            po7[:],
                            ctxT[:, 7, mi * P:(mi + 1) * P],
                            wo[:, 7, half * 512:(half + 1) * 512],
                            start=True, stop=True,
                        )
                        ot = outp.tile([P, 512], F32, tag="ot")
                        nc.vector.tensor_tensor(
                            ot[:], po7[:], partials[ci][:], add)
                        nc.sync.dma_start(
                            out_s.ap().rearrange("(mm p) d -> p mm d", p=P)[
                                :, mi, half * 512:(half + 1) * 512],
                            ot[:])

    nc.compile()
    return nc


def _prep_inputs(hidden_states, key_value_states, q_weight, q_bias,
                 kv_weight, kv_bias, out_weight, out_bias):
    f32 = np.float32
    bf16 = ml_dtypes.bfloat16
    hid = np.asarray(hidden_states, f32).reshape(B * LQ, D).astype(bf16)
    kv = np.asarray(key_value_states, f32).reshape(B * LK, D).astype(bf16)
    hidT_cores = [np.ascontiguousarray(hid[c * TQ:(c + 1) * TQ].T)
                  for c in range(NCORES)]
    kvT_batches = [np.ascontiguousarray(kv[b * LK:(b + 1) * LK].T)
                   for b in range(B)]
    scale = f32(1.0 / 8.0)

    # de-interleave kv rows: row e <-> (h=e//128, j=(e%128)//64, d=e%64)
    e = np.arange(2 * D)
    kmask = (e % 128) < 64
    kidx, vidx = e[kmask], e[~kmask]
    kvw = np.asarray(kv_weight, f32)
    kvb = np.asarray(kv_bias, f32)

    shared = {
        "wq_t": np.ascontiguousarray(
            (np.asarray(q_weight, f32) * scale).T).astype(bf16),
        "wk_t": np.ascontiguousarray(kvw[kidx].T).astype(bf16),
        "wv_t": np.ascontiguousarray(kvw[vidx].T).astype(bf16),
        "wo_t": np.ascontiguousarray(np.asarray(out_weight, f32).T).astype(bf16),
        "qb": np.ascontiguousarray(np.asarray(q_bias, f32) * scale),
        "kb": np.ascontiguousarray(kvb[kidx]),
        "vb": np.ascontiguousarray(kvb[vidx]),
        "ob": np.ascontiguousarray(np.asarray(out_bias, f32)),
    }
    in_maps = []
    for c in range(NCORES):
        b = c // 2
        m = dict(shared)
        m["hid_s"] = hidT_cores[c]
        m["kv_s"] = kvT_batches[b]
        in_maps.append(m)
    return in_maps


def kernel(hidden_states, key_value_states, q_weight, q_bias,
           kv_weight, kv_bias, out_weight, out_bias, _trace=False):
    if "nc" not in _CACHE:
        _CACHE["nc"] = _build_core_program()
    nc = _CACHE["nc"]
    in_maps = _prep_inputs(hidden_states, key_value_states, q_weight, q_bias,
                           kv_weight, kv_bias, out_weight, out_bias)
    res = bass_utils.run_bass_kernel_spmd(
        nc, in_maps, core_ids=list(range(NCORES)), trace=_trace)
    _CACHE["last_result"] = res
    out = np.concatenate([r["out_s"] for r in res.results], axis=0)
    return out.reshape(B, LQ, D)


# revision 28
# speedup vs baseline: 1.0608x; 1.0163x over previous
"""Trainium2 Bass kernel for nn_BartCrossAttention (B=4, L=1024, D=1024, H=16, HD=64).

Sharding: 8 cores; core c handles query tokens [512c, 512c+512) (batch b = c//2).
Each core recomputes K/V projections for its *whole* batch (1024 kv tokens) so no
collective is needed (a pairwise K/V exchange was measured at ~90us through this
stack's DRAM AllGather — far too slow to hide); the host slices inputs per core
and concatenates outputs.

Key design points (vs the fp32r v1 baseline, 264.6us -> ~200-205us):
- Everything bf16 (PE rate is 1 cycle/row for bf16 and for fp32r at free>=256,
  but bf16 halves DMA, SBUF, and DVE traffic); fp32 PSUM accumulation and an
  fp32 softmax chain keep rel err ~6e-3 (gate 2e-2).
- hid^T / kv^T are transposed on the HOST, so the PE does no transposes; the
  kvT/wv DMAs are split in halves and interleaved so the V projection starts
  as soon as the first halves land.
- All four weights live in SBUF whole; K/Q projections for pair p+1 are
  interleaved into pair p's attention t-loop.
- Fused softmax denominators ride the ctx matmul as a 65th lhsT column of ones.
  Per-pair normalization runs fully in SBUF: DVE copies the two denominator
  rows out of PSUM, DVE reciprocal (6.5us on a 1-partition row, but nothing
  the PE waits on sits behind it: the K/Q PSUM evicts were moved to ACT as
  Identity+per-partition-bias), gpsimd broadcasts down 64 partitions, DVE
  multiplies fused with the PSUM evict. psctx has 4 banks so consecutive
  pairs double-buffer and the chain hides under the next pair's attention.
- PSUM: main loop = 4 ctx banks + 4 score/proj banks (per-tag pools); the
  epilogue reuses the score pool (a fresh pool would barrier on the last norm
  chain). Out-projection runs fj=0..6 of all 8 output chains first (partials
  evicted to SBUF with the bias folded in, freeing banks), then the 8 fj=7
  rank-128 updates after the last pair's norm resolves.
- exp runs per [128,512] score half on ACT, double-buffered against the PE.
"""
import sys

for _p in ("/opt/trn_rl_repo",):
    if _p not in sys.path:
        sys.path.insert(0, _p)

import numpy as np
import ml_dtypes

import concourse.bass as bass
import concourse.mybir as mybir
import concourse.tile as tile
from concourse import bacc
import concourse.bass_utils as bass_utils

F32 = mybir.dt.float32
BF16 = mybir.dt.bfloat16

P = 128
D = 1024        # model dim
H = 16          # heads
NCORES = 8
TQ = 512        # query tokens per core
LK = 1024       # kv tokens per batch
B, LQ = 4, 1024

_CACHE = {}


def _build_core_program():
    nc = bacc.Bacc("TRN2", target_bir_lowering=False, debug=False,
                   num_devices=NCORES)

    hid_s = nc.dram_tensor("hid_s", [D, TQ], BF16, kind="ExternalInput")
    kv_s = nc.dram_tensor("kv_s", [D, LK], BF16, kind="ExternalInput")
    wq_t = nc.dram_tensor("wq_t", [D, D], BF16, kind="ExternalInput")
    wk_t = nc.dram_tensor("wk_t", [D, D], BF16, kind="ExternalInput")
    wv_t = nc.dram_tensor("wv_t", [D, D], BF16, kind="ExternalInput")
    wo_t = nc.dram_tensor("wo_t", [D, D], BF16, kind="ExternalInput")
    qb_d = nc.dram_tensor("qb", [D], F32, kind="ExternalInput")
    kb_d = nc.dram_tensor("kb", [D], F32, kind="ExternalInput")
    vb_d = nc.dram_tensor("vb", [D], F32, kind="ExternalInput")
    ob_d = nc.dram_tensor("ob", [D], F32, kind="ExternalInput")
    out_s = nc.dram_tensor("out_s", [TQ, D], F32, kind="ExternalOutput")

    Exp = mybir.ActivationFunctionType.Exp
    Ident = mybir.ActivationFunctionType.Identity
    add = mybir.AluOpType.add
    mult = mybir.AluOpType.mult

    with tile.TileContext(nc) as tc:
        with (
            tc.tile_pool(name="setup", bufs=1) as setup,
            tc.tile_pool(name="big", bufs=1) as big,
            tc.tile_pool(name="attn", bufs=6) as attnp,
            tc.tile_pool(name="wfull", bufs=1) as wfull,
            tc.tile_pool(name="rb", bufs=1) as rbp,
            tc.tile_pool(name="outp", bufs=4) as outp,
            tc.tile_pool(name="partp", bufs=8) as partp,
        ):
            # ---- setup: biases ----
            qb_sb = setup.tile([P, 8], F32, tag="qb")
            nc.sync.dma_start(qb_sb[:], qb_d.ap().rearrange("(o p) -> p o", p=P))
            kb_sb = setup.tile([P, 8], F32, tag="kb")
            nc.sync.dma_start(kb_sb[:], kb_d.ap().rearrange("(o p) -> p o", p=P))
            vbB = setup.tile([P, D], F32, tag="vbB")
            obB = setup.tile([P, D], F32, tag="obB")

            def load_w_full(dram, tag):
                # [D, D] -> tile [128, 8, 1024]: w[di*128+p, o] at [p, di, o]
                t = wfull.tile([P, 8, D], BF16, tag=tag)
                nc.sync.dma_start(
                    t[:], dram.ap().rearrange("(dd p) o -> p dd o", p=P))
                return t

            # ---- persistent big tiles ----
            KT = big.tile([P, 8, LK], BF16, tag="KT")        # K^T [1024, 1024]
            v65 = big.tile([P, 8, H * 65], BF16, tag="v65")  # V+ones [1024,1040]
            qT = big.tile([P, 8, TQ], BF16, tag="qT")        # Q^T [1024, 512]
            ctxT = big.tile([P, 8, TQ], BF16, tag="ctxT")    # ctx^T [1024, 512]

            # ones columns of v65 (col 64 of each head block)
            onesF = setup.tile([P, P], F32, tag="onesF")
            nc.gpsimd.memset(onesF[:], 1.0)
            nc.vector.tensor_copy(
                v65[:].rearrange("p t (h x) -> p t h x", x=65)[:, :, :, 64:65],
                onesF[:].rearrange("p (t h x) -> p t h x", t=8, h=16))

            with tc.tile_pool(name="xTp", bufs=1) as xTp:
                kvT = xTp.tile([P, 8, LK], BF16, tag="kvT")   # kv^T [D, 1024]
                hidT = xTp.tile([P, 8, TQ], BF16, tag="hidT")  # hid^T [1024,512]

                with (
                    tc.tile_pool(name="browp", bufs=2) as browp,
                    tc.tile_pool(name="wvpool", bufs=1) as wvpool,
                    tc.tile_pool(name="psv", bufs=2, space="PSUM") as psv,
                ):
                    # bias rows -> broadcast
                    vb_row = browp.tile([1, D], F32, tag="brow")
                    nc.sync.dma_start(vb_row[:], vb_d.ap()[None, :])
                    nc.gpsimd.partition_broadcast(vbB[:], vb_row[:])
                    ob_row = browp.tile([1, D], F32, tag="brow")
                    nc.sync.dma_start(ob_row[:], ob_d.ap()[None, :])
                    nc.gpsimd.partition_broadcast(obB[:], ob_row[:])

                    # ---- prologue: kv^T and hid^T arrive pre-transposed
                    # from the host; kvT token-halves interleave with the wv
                    # halves on the DMA queue so V proj starts as soon as
                    # the first halves of each land ----
                    kvt_ap = kv_s.ap().rearrange("(dd p) t -> p dd t", p=P)
                    wv_ap = wv_t.ap().rearrange("(dd p) o -> p dd o", p=P)
                    wv_halves = []
                    # first V-proj matmul only needs kv tokens 0:256 and the
                    # first contraction half of wv's first column-half, so
                    # split those DMAs fine-grained to shrink the gate.
                    nc.sync.dma_start(kvT[:, :, 0:256], kvt_ap[:, :, 0:256])
                    wvh = wvpool.tile([P, 8, 512], BF16, tag="wv")
                    nc.sync.dma_start(wvh[:, 0:4, :], wv_ap[:, 0:4, 0:512])
                    nc.sync.dma_start(wvh[:, 4:8, :], wv_ap[:, 4:8, 0:512])
                    wv_halves.append(wvh)
                    nc.sync.dma_start(kvT[:, :, 256:512],
                                      kvt_ap[:, :, 256:512])
                    # hidT next: Q-proj for pair 0 fires right after the V
                    # projection and must not wait on the activation stream
                    nc.sync.dma_start(
                        hidT[:],
                        hid_s.ap().rearrange("(dd p) t -> p dd t", p=P))
                    nc.sync.dma_start(kvT[:, :, 512:768],
                                      kvt_ap[:, :, 512:768])
                    wvh = wvpool.tile([P, 8, 512], BF16, tag="wv")
                    nc.sync.dma_start(wvh[:], wv_ap[:, :, 512:1024])
                    wv_halves.append(wvh)
                    nc.sync.dma_start(kvT[:, :, 768:1024],
                                      kvt_ap[:, :, 768:1024])

                    for half in range(2):             # v-col half
                        wv = wv_halves[half]
                        for ti in range(8):           # kv token tile
                            pp = psv.tile([P, 512], F32, tag="pp")
                            for di in range(8):
                                nc.tensor.matmul(
                                    pp[:],
                                    kvT[:, di, ti * P:(ti + 1) * P],
                                    wv[:, di, :],
                                    start=(di == 0), stop=(di == 7),
                                )
                            dst = v65[:].rearrange(
                                "p t (h x) -> p t h x", x=65)[
                                :, ti, half * 8:(half + 1) * 8, 0:64]
                            nc.vector.tensor_tensor(
                                dst, pp[:],
                                vbB[:, half * 512:(half + 1) * 512], add)

                # ---- main loop: per head-pair K/Q projection + attention ----
                with (
                    tc.tile_pool(name="psctx", bufs=4, space="PSUM") as psctx,
                    tc.tile_pool(name="pssc", bufs=4, space="PSUM") as pssc,
                ):
                    psk = pssc
                    # pair-0/1 columns of wk/wq land first so the pair-0
                    # projections never wait on the bulk of the weights
                    def load_w_split(dram, tag):
                        t = wfull.tile([P, 8, D], BF16, tag=tag)
                        ap = dram.ap().rearrange("(dd p) o -> p dd o", p=P)
                        nc.sync.dma_start(t[:, :, 0:256], ap[:, :, 0:256])
                        return t, ap

                    wk, wk_ap = load_w_split(wk_t, "wk")
                    wq, wq_ap = load_w_split(wq_t, "wq")
                    nc.sync.dma_start(wk[:, :, 256:1024], wk_ap[:, :, 256:1024])
                    nc.sync.dma_start(wq[:, :, 256:1024], wq_ap[:, :, 256:1024])
                    wo = load_w_full(wo_t, "wo")

                    def emit_kproj(hp, nk):
                        pp = psk.tile([P, 512], F32, tag="sc",
                                      name=f"ppk{hp}_{nk}")
                        for di in range(8):
                            nc.tensor.matmul(
                                pp[:],
                                wk[:, di, hp * P:(hp + 1) * P],
                                kvT[:, di, nk * 512:(nk + 1) * 512],
                                start=(di == 0), stop=(di == 7),
                            )
                        nc.scalar.activation(
                            KT[:, hp, nk * 512:(nk + 1) * 512], pp[:],
                            Ident, bias=kb_sb[:, hp:hp + 1])

                    def emit_qproj(hp):
                        pq = psk.tile([P, 512], F32, tag="sc",
                                      name=f"ppq{hp}")
                        for di in range(8):
                            nc.tensor.matmul(
                                pq[:],
                                wq[:, di, hp * P:(hp + 1) * P],
                                hidT[:, di, :],
                                start=(di == 0), stop=(di == 7),
                            )
                        nc.scalar.activation(qT[:, hp, :], pq[:],
                                             Ident, bias=qb_sb[:, hp:hp + 1])

                    # pair 0 projections up front (t<4 only needs nk=0)
                    emit_kproj(0, 0)
                    emit_qproj(0)
                    emit_kproj(0, 1)

                    for hp in range(8):
                        nxt = hp + 1
                        ctx_ps = [psctx.tile([65, 512], F32, tag="ctx",
                                             name=f"ctx{hp}_{i}")
                                  for i in range(2)]

                        def emit_ctx(t, ats):
                            for hh in range(2):
                                h = 2 * hp + hh
                                nc.tensor.matmul(
                                    ctx_ps[hh][:],
                                    v65[:, t, h * 65:(h + 1) * 65],
                                    ats[hh][:],
                                    start=(t == 0), stop=(t == 7),
                                )

                        # software pipeline: scores/exp for tile t issue
                        # before the ctx matmuls of tile t-1, so each exp
                        # hides behind the next score matmul's streaming.
                        prev = None
                        for t in range(8):
                            ats = []
                            for hh in range(2):
                                lo = 64 * hh
                                sc = pssc.tile([P, 512], F32, tag="sc",
                                               name=f"sc{hp}_{t}_{hh}")
                                nc.tensor.matmul(
                                    sc[:],
                                    KT[lo:lo + 64, hp, t * P:(t + 1) * P],
                                    qT[lo:lo + 64, hp, :],
                                    start=True, stop=True,
                                )
                                at = attnp.tile([P, 512], BF16, tag="at")
                                nc.scalar.activation(at[:], sc[:], Exp)
                                ats.append(at)
                            if prev is not None:
                                emit_ctx(t - 1, prev)
                            prev = ats
                            if nxt < 8:
                                if t == 1:
                                    emit_kproj(nxt, 0)
                                elif t == 3:
                                    emit_kproj(nxt, 1)
                                elif t == 5:
                                    emit_qproj(nxt)
                        emit_ctx(7, prev)
                        # per-pair softmax normalization, all in SBUF:
                        # DVE copies the denominator rows out of PSUM, ACT
                        # takes the reciprocal (free-size bound), gpsimd
                        # broadcasts down 64 partitions, DVE multiplies
                        # fused with the PSUM evict.
                        # DVE reciprocal time scales with free size only, so
                        # stacking the two denominator rows on partitions 0
                        # and 64 of one tile halves it (3.3us vs 6.5); with
                        # the K/Q evicts on ACT nothing the PE waits on sits
                        # behind it, and psctx bufs=4 gives the chain two
                        # pair-periods of slack.
                        s65 = rbp.tile([65, 512], F32, tag="srow",
                                       name=f"srow{hp}")
                        nc.vector.tensor_copy(s65[0:1, :],
                                              ctx_ps[0][64:65, :])
                        nc.vector.tensor_copy(s65[64:65, :],
                                              ctx_ps[1][64:65, :])
                        nc.vector.reciprocal(s65[:], s65[:])
                        rowO = rbp.tile([1, 512], F32, tag="rowO",
                                        name=f"rowO{hp}")
                        nc.vector.tensor_copy(rowO[:], s65[64:65, :])
                        rcpE = rbp.tile([64, 512], F32, tag="rcpE",
                                        name=f"rcpE{hp}")
                        rcpO = rbp.tile([64, 512], F32, tag="rcpO",
                                        name=f"rcpO{hp}")
                        nc.gpsimd.partition_broadcast(rcpE[:], s65[0:1, :])
                        nc.gpsimd.partition_broadcast(rcpO[:], rowO[:])
                        nc.vector.tensor_tensor(
                            ctxT[0:64, hp, :], ctx_ps[0][0:64, :],
                            rcpE[:], mult)
                        nc.vector.tensor_tensor(
                            ctxT[64:128, hp, :], ctx_ps[1][0:64, :],
                            rcpO[:], mult)

                    # ---- epilogue: out projection (inside the main PSUM
                    # scope: po accumulators come from pssc, whose banks free
                    # fast — a fresh pool here would barrier on the last
                    # pair's norm chain). Each chain's fj=7 lhsT (pair 7's
                    # ctxT) is only ready after that ~10us chain; running
                    # fj=0..6 of several chains ahead of any fj=7 hides it.
                    chains = [(half, mi) for half in range(2)
                              for mi in range(4)]

                    # fj=0..6 of every chain accumulates first (partials
                    # evicted to SBUF with the bias folded in, freeing the
                    # PSUM bank); the fj=7 rank-128 updates and final adds
                    # run after the last pair's norm chain resolves.
                    partials = []
                    for ci, (half, mi) in enumerate(chains):
                        po = pssc.tile([P, 512], F32, tag="sc",
                                       name=f"po{ci}")
                        for fj in range(7):
                            nc.tensor.matmul(
                                po[:],
                                ctxT[:, fj, mi * P:(mi + 1) * P],
                                wo[:, fj, half * 512:(half + 1) * 512],
                                start=(fj == 0), stop=(fj == 6),
                            )
                        part = partp.tile([P, 512], BF16, tag="part",
                                          name=f"part{ci}")
                        nc.vector.tensor_tensor(
                            part[:], po[:],
                            obB[:, half * 512:(half + 1) * 512], add)
                        partials.append(part)
                    for ci, (half, mi) in enumerate(chains):
                        po7 = pssc.tile([P, 512], F32, tag="sc",
                                        name=f"po7_{ci}")
                        nc.tensor.matmul(
                            po7[:],
                            ctxT[:, 7, mi * P:(mi + 1) * P],
                            wo[:, 7, half * 512:(half + 1) * 512],
                            start=True, stop=True,
                        )
                        ot = outp.tile([P, 512], F32, tag="ot")
                        nc.vector.tensor_tensor(
                            ot[:], po7[:], partials[ci][:], add)
                        nc.sync.dma_start(
                            out_s.ap().rearrange("(mm p) d -> p mm d", p=P)[
                                :, mi, half * 512:(half + 1) * 512],
                            ot[:])

    nc.compile()
    return nc


def _prep_inputs(hidden_states, key_value_states, q_weight, q_bias,
                 kv_weight, kv_bias, out_weight, out_bias):
    f32 = np.float32
    bf16 = ml_dtypes.bfloat16
    hid = np.asarray(hidden_states, f32).reshape(B * LQ, D).astype(bf16)
    kv = np.asarray(key_value_states, f32).reshape(B * LK, D).astype(bf16)
    hidT_cores = [np.ascontiguousarray(hid[c * TQ:(c + 1) * TQ].T)
                  for c in range(NCORES)]
    kvT_batches = [np.ascontiguousarray(kv[b * LK:(b + 1) * LK].T)
                   for b in range(B)]
    scale = f32(1.0 / 8.0)

    # de-interleave kv rows: row e <-> (h=e//128, j=(e%128)//64, d=e%64)
    e = np.arange(2 * D)
    kmask = (e % 128) < 64
    kidx, vidx = e[kmask], e[~kmask]
    kvw = np.asarray(kv_weight, f32)
    kvb = np.asarray(kv_bias, f32)

    shared = {
        "wq_t": np.ascontiguousarray(
            (np.asarray(q_weight, f32) * scale).T).astype(bf16),
        "wk_t": np.ascontiguousarray(kvw[kidx].T).astype(bf16),
        "wv_t": np.ascontiguousarray(kvw[vidx].T).astype(bf16),
        "wo_t": np.ascontiguousarray(np.asarray(out_weight, f32).T).astype(bf16),
        "qb": np.ascontiguousarray(np.asarray(q_bias, f32) * scale),
        "kb": np.ascontiguousarray(kvb[kidx]),
        "vb": np.ascontiguousarray(kvb[vidx]),
        "ob": np.ascontiguousarray(np.asarray(out_bias, f32)),
    }
    in_maps = []
    for c in range(NCORES):
        b = c // 2
        m = dict(shared)
        m["hid_s"] = hidT_cores[c]
        m["kv_s"] = kvT_batches[b]
        in_maps.append(m)
    return in_maps


def kernel(hidden_states, key_value_states, q_weight, q_bias,
           kv_weight, kv_bias, out_weight, out_bias, _trace=False):
    if "nc" not in _CACHE:
        _CACHE["nc"] = _build_core_program()
    nc = _CACHE["nc"]
    in_maps = _prep_inputs(hidden_states, key_value_states, q_weight, q_bias,
                           kv_weight, kv_bias, out_weight, out_bias)
    res = bass_utils.run_bass_kernel_spmd(
        nc, in_maps, core_ids=list(range(NCORES)), trace=_trace)
    _CACHE["last_result"] = res
    out = np.concatenate([r["out_s"] for r in res.results], axis=0)
    return out.reshape(B, LQ, D)
